# revision 32
# baseline (speedup 1.0000x reference)
# Trainium2 Bass kernel for the recursive tree autoencoder (gnn_message_passing).
#
# Sharding: subtree partitioning. Each of the 8 cores owns a contiguous block of
# 16384 leaves and all their ancestors up to its level-14 subtree root, so tree
# levels 0..13 (both encode and decode) are fully core-local. The 8 subtree-root
# features are AllGathered (8 x 1KB) and the top 3 tree levels are computed
# replicated on every core.
#
# Observations exploited:
#   * The decoder overwrites Feat[l]/Feat[r] before ever reading them, so the
#     only encoder output the decoder consumes is the root feature. No
#     intermediate features ever need to touch HBM: per-level h/c arrays live in
#     SBUF (feature-dim on partitions, nodes on the free dim).
#   * left/right children of a level are even/odd node indices, so all tree
#     gathers/scatters are stride-2 access patterns - no indirect DMA at all.
#   * X_ab (absolute frame propagation) depends only on the input X, never on
#     network outputs, so it is precomputed on the host and shipped as an input.
#   * The loss is a scalar: head outputs are computed (merges-on-partitions,
#     6-on-free) so mkP + squared error run at full 128-lane utilization with
#     ScalarE Square+accum_out; the host does the final tiny weighted reduce.

import math

import numpy as np

NF = 256
HD = 128  # half feature dim
NCORES = 8
FULL_LEAVES = 131072
FULL_LPC = FULL_LEAVES // NCORES  # leaves per core

F32 = np.float32


def _tree_meta(lpc):
    loclev = int(math.log2(lpc))
    assert (1 << loclev) == lpc
    # xin: children X for local merge levels 0..loclev-1, level-major
    xoff = []
    off = 0
    for k in range(loclev):
        xoff.append(off)
        off += lpc >> k
    xin_cols = off  # 2*lpc - 2
    xin_pad = xin_cols + (xin_cols % 2)
    # xab: parent X_ab for local levels (parents at node-levels 1..loclev)
    xpoff = {}
    off = 0
    for k in range(1, loclev + 1):
        xpoff[k] = off
        off += lpc >> k
    xab_cols = off  # lpc - 1
    xab_pad = xab_cols + (xab_cols % 2)
    # xloss: per merge-level slabs of 5*ceil(m/128) cols (block-major)
    xloff = []
    off = 0
    for k in range(loclev):
        m = lpc >> (k + 1)
        xloff.append(off)
        off += 5 * ((m + 127) // 128)
    xloss_cols = off
    # packed X layouts: slot s covers concat-cols [s*W, (s+1)*W) on
    # partitions 32*s..32*s+5 (matmul operands may only start at partition
    # 0/32/64, so at most 3 slots). The slot boundaries are multiples of
    # 512, so no (level, 512-sub) slice ever crosses a slot.
    if lpc >= 2048:
        xw = 3 * lpc // 4
        xaw = lpc // 2
    else:
        xw = xin_pad
        xaw = xab_pad
    return (loclev, xoff, xin_pad, xpoff, xab_pad, xloff, xloss_cols,
            xw, xaw)


def _pack_x(arr5, W):
    """(5, cols) -> packed (32*(nslots-1)+5, W) with slot s at partition
    32*s covering cols [s*W, (s+1)*W)."""
    cols = arr5.shape[1]
    nslots = (cols + W - 1) // W
    out = np.zeros((32 * (nslots - 1) + 5, W), F32)
    for s in range(nslots):
        c0 = s * W
        cw = min(W, cols - c0)
        out[32 * s:32 * s + 5, :cw] = arr5[:, c0:c0 + cw]
    return out


# ---------------------------------------------------------------------------
# program builder
# ---------------------------------------------------------------------------

def build_program(lpc, subw=512, chw=512, acc_cols=512, mm_dt=None):
    """Build the SPMD Bacc program for `lpc` leaves per core.

    Returns (nc, col_map) where col_map[i] = tree-level of loss column i.
    """
    import concourse.bacc as bacc
    import concourse.mybir as mybir
    import concourse.tile as tile
    from contextlib import ExitStack

    f32 = mybir.dt.float32
    AF = mybir.ActivationFunctionType
    ALU = mybir.AluOpType

    (loclev, xoff, xin_pad, xpoff, xab_pad, xloff, xloss_cols,
     xw, xaw) = _tree_meta(lpc)
    chw = min(chw, lpc)
    assert chw >= 8 and lpc % chw == 0
    xr_parts = 32 * ((xin_pad + xw - 1) // xw - 1) + 5
    xa_parts = 32 * ((xab_pad + xaw - 1) // xaw - 1) + 5

    nc = bacc.Bacc("TRN2", target_bir_lowering=False, debug=False,
                   num_devices=NCORES)

    def din(name, shape):
        return nc.dram_tensor(name, list(shape), f32, kind="ExternalInput").ap()

    d_fh0 = din("fh0", (HD, lpc))
    d_fc0 = din("fc0", (HD, lpc))
    d_xraw = din("xraw", (xr_parts, xw))
    d_xab = din("xab", (xa_parts, xaw))
    d_xtraw = din("xtraw", (5, 16))
    d_xtab = din("xtab", (5, 16))
    d_xlsl = din("xlsl", (HD, xloss_cols))
    d_xlsr = din("xlsr", (HD, xloss_cols))
    d_xtlsl = din("xtlsl", (HD, 16))
    d_xtlsr = din("xtlsr", (HD, 16))
    d_ewih = din("ewih", (101, 4 * HD))
    d_ewhh = din("ewhh", (HD, 4 * HD))
    d_eb = din("eb", (HD, 4))
    d_fha = din("fha", (HD, 2 * NF))
    d_fhb = din("fhb", (HD, 2 * NF))
    d_fhbias = din("fhbias", (HD, 4))
    d_dwx = din("dwx", (101, 4 * NF))
    d_dwfh = din("dwfh", (HD, 4 * NF))
    d_dwfl = din("dwfl", (HD, 4 * NF))
    d_dha = din("dha", (HD, 4 * NF))
    d_dhb = din("dhb", (HD, 4 * NF))
    d_db = din("db", (HD, 8))
    d_flw = din("flw", (HD, 6))
    d_frw = din("frw", (HD, 6))
    d_hbias = din("hbias", (1, 12))
    d_mask = din("mask", (HD, 8))

    d_out = nc.dram_tensor("acc_out", [HD, acc_cols], f32,
                           kind="ExternalOutput").ap()

    col_map = []

    def alloc_col(level):
        col_map.append(level)
        assert len(col_map) <= acc_cols
        return len(col_map) - 1

    with ExitStack() as stk:
        tc = stk.enter_context(tile.TileContext(nc))
        cpool = stk.enter_context(tc.tile_pool(name="const", bufs=1))
        psum = stk.enter_context(tc.tile_pool(name="psum", bufs=6,
                                              space="PSUM"))
        work = stk.enter_context(tc.tile_pool(name="work", bufs=2))
        dram = stk.enter_context(tc.tile_pool(name="dram", bufs=1,
                                              space="DRAM"))

        def cload(dap, shape, name):
            t = cpool.tile(list(shape), f32, tag=name, name=name)
            nc.sync.dma_start(t[:], dap)
            return t

        xtraw = cload(d_xtraw, (5, 16), "xtraw")
        xtab = cload(d_xtab, (5, 16), "xtab")
        xlsl = cload(d_xlsl, (HD, xloss_cols), "xlsl")
        xlsr = cload(d_xlsr, (HD, xloss_cols), "xlsr")
        xtlsl = cload(d_xtlsl, (HD, 16), "xtlsl")
        xtlsr = cload(d_xtlsr, (HD, 16), "xtlsr")
        ewih = cload(d_ewih, (101, 4 * HD), "ewih")
        ewhh = cload(d_ewhh, (HD, 4 * HD), "ewhh")
        eb = cload(d_eb, (HD, 4), "eb")
        fha = cload(d_fha, (HD, 2 * NF), "fha")
        fhb = cload(d_fhb, (HD, 2 * NF), "fhb")
        fhbias = cload(d_fhbias, (HD, 4), "fhbias")
        dwx = cload(d_dwx, (101, 4 * NF), "dwx")
        dwfh = cload(d_dwfh, (HD, 4 * NF), "dwfh")
        dwfl = cload(d_dwfl, (HD, 4 * NF), "dwfl")
        dha = cload(d_dha, (HD, 4 * NF), "dha")
        dhb = cload(d_dhb, (HD, 4 * NF), "dhb")
        db = cload(d_db, (HD, 8), "db")
        flw = cload(d_flw, (HD, 6), "flw")
        frw = cload(d_frw, (HD, 6), "frw")
        hbias = cload(d_hbias, (1, 12), "hbias")
        mask = cload(d_mask, (HD, 8), "mask")

        ones = cpool.tile([1, HD], f32, tag="ones", name="ones")
        nc.vector.memset(ones[:], 1.0)
        acc = cpool.tile([HD, acc_cols], f32, tag="acc", name="acc")
        nc.vector.memset(acc[:], 0.0)

        mdt = None if mm_dt is None else getattr(mybir.dt, mm_dt)

        def mm(ps, lhsT, rhs, start, stop):
            if mdt is not None:
                lhsT = lhsT.bitcast(mdt)
                rhs = rhs.bitcast(mdt)
            nc.tensor.matmul(ps, lhsT, rhs, start=start, stop=stop)

        # ---------------- encoder level ----------------
        def emit_enc_level(h_ap, c_ap, xf, n, hout, cout):
            """One encoder merge level: n children rows -> n/2 parents.
            xf(s0, sw) -> (5, sw) AP of the children's X."""
            for s0 in range(0, n, subw):
                sw = min(subw, n - s0)
                gts = []
                for g in range(4):
                    ps = psum.tile([HD, sw], f32, tag="ps", name="eps")
                    mm(ps[:], ewhh[:, g * HD:(g + 1) * HD],
                       h_ap[:, s0:s0 + sw], start=True, stop=False)
                    xap, xs = xf(s0, sw)
                    mm(ps[:], ewih[32 * xs:32 * xs + 5,
                                   g * HD:(g + 1) * HD],
                       xap, start=False, stop=True)
                    fn = AF.Tanh if g == 2 else AF.Sigmoid
                    gt = work.tile([HD, sw], f32, tag=f"g{g}", name=f"eg{g}")
                    nc.scalar.activation(gt[:], ps[:], fn, bias=eb[:, g:g + 1])
                    gts.append(gt)
                ti, tf, tg, to = gts
                t1 = work.tile([HD, sw], f32, tag="t1", name="et1")
                nc.vector.tensor_mul(t1[:], tf[:], c_ap[:, s0:s0 + sw])
                t2 = work.tile([HD, sw], f32, tag="t2", name="et2")
                nc.vector.tensor_mul(t2[:], ti[:], tg[:])
                c2 = work.tile([HD, sw], f32, tag="c2s", name="ec2")
                nc.vector.tensor_add(c2[:], t1[:], t2[:])
                tc2 = work.tile([HD, sw], f32, tag="tct", name="etc2")
                nc.scalar.activation(tc2[:], c2[:], AF.Tanh)
                h2 = work.tile([HD, sw], f32, tag="h2s", name="eh2")
                nc.vector.tensor_mul(h2[:], to[:], tc2[:])
                p0 = s0 // 2
                pw = sw // 2
                nc.vector.tensor_add(hout[:, p0:p0 + pw],
                                     h2[:, 0::2], h2[:, 1::2])
                nc.vector.tensor_add(cout[:, p0:p0 + pw],
                                     c2[:, 0::2], c2[:, 1::2])

        # ---------------- decoder step ----------------
        def emit_dec_step(fh, fc, xpf, k, pcol0, P, out_fh, out_fc,
                          xls_pair, xl_base, last):
            """Decoder step at merge-level k: P parents -> 2P children.

            fh/fc: (HD, P) APs over the parents' split features.
            xpf(s0, sw): (5, sw) AP of parent X_ab values.
            pcol0: parent offset within the level (for xloss block indexing).
            out_fh/out_fc: (HD, 2P) APs for children features (None if last).
            xls_pair: (left, right) xloss const tiles; xl_base: level slab col.
            """
            for s0 in range(0, P, subw):
                sw = min(subw, P - s0)
                zt = []
                for g in range(4):
                    ps = psum.tile([HD, sw], f32, tag="ps", name="zps")
                    mm(ps[:], fha[:, g * HD:(g + 1) * HD],
                       fh[:, s0:s0 + sw], start=True, stop=False)
                    mm(ps[:], fhb[:, g * HD:(g + 1) * HD],
                       fc[:, s0:s0 + sw], start=False, stop=True)
                    z = work.tile([HD, sw], f32, tag=f"z{g}", name=f"z{g}")
                    nc.scalar.activation(z[:], ps[:], AF.Relu,
                                         bias=fhbias[:, g:g + 1])
                    zt.append(z)
                gts = []
                for g in range(8):
                    ps = psum.tile([HD, sw], f32, tag="ps", name="gps")
                    sl = slice(g * HD, (g + 1) * HD)
                    xap, xs = xpf(s0, sw)
                    mm(ps[:], dwx[32 * xs:32 * xs + 5, sl], xap,
                       start=True, stop=False)
                    mm(ps[:], dwfh[:, sl], fh[:, s0:s0 + sw],
                       start=False, stop=False)
                    mm(ps[:], dwfl[:, sl], fc[:, s0:s0 + sw],
                       start=False, stop=False)
                    mm(ps[:], dha[:, sl], zt[0][:], start=False, stop=False)
                    mm(ps[:], dhb[:, sl], zt[1][:], start=False, stop=True)
                    fn = AF.Tanh if g in (4, 5) else AF.Sigmoid
                    gt = work.tile([HD, sw], f32, tag=f"g{g}", name=f"dg{g}")
                    nc.scalar.activation(gt[:], ps[:], fn, bias=db[:, g:g + 1])
                    gts.append(gt)
                for half in (0, 1):
                    ig = gts[0 + half]
                    fg = gts[2 + half]
                    gg = gts[4 + half]
                    og = gts[6 + half]
                    c0h = zt[2 + half]
                    t1 = work.tile([HD, sw], f32, tag="t1", name="dt1")
                    nc.vector.tensor_mul(t1[:], fg[:], c0h[:])
                    t2 = work.tile([HD, sw], f32, tag="t2", name="dt2")
                    nc.vector.tensor_mul(t2[:], ig[:], gg[:])
                    c2 = work.tile([HD, sw], f32, tag="c2s", name="dc2")
                    nc.vector.tensor_add(c2[:], t1[:], t2[:])
                    tch = work.tile([HD, sw], f32, tag="tct", name="dtc")
                    nc.scalar.activation(tch[:], c2[:], AF.Tanh)
                    ho = work.tile([HD, sw], f32, tag="h2s", name="dho")
                    nc.vector.tensor_mul(ho[:], og[:], tch[:])
                    if not last:
                        co = 2 * s0 + half
                        ce = co + 2 * sw - 1  # exclusive stop, stride-2 safe
                        nc.gpsimd.tensor_copy(out_fh[:, co:ce:2], ho[:])
                        nc.gpsimd.tensor_copy(out_fc[:, co:ce:2], c2[:])
                    # ---- head + loss, merges-on-partitions layout ----
                    nj = max(1, sw // HD)
                    pw = min(HD, sw)
                    hw = flw if half == 0 else frw
                    hb = hbias[:, 6 * half:6 * half + 6]
                    hp = psum.tile([HD, 6 * nj], f32, tag="hps", name="hps",
                                   bufs=2)
                    for j in range(nj):
                        mm(hp[0:pw, 6 * j:6 * j + 6],
                           ho[:, j * HD:j * HD + pw], hw[:, 0:6],
                           start=True, stop=False)
                        mm(hp[0:pw, 6 * j:6 * j + 6], ones[:, 0:pw], hb,
                           start=False, stop=True)
                    lp = work.tile([HD, 6 * nj], f32, tag="lp", name="lp")
                    hp3 = hp[:].rearrange("p (j c) -> p j c", c=6)
                    lp3 = lp[:].rearrange("p (j c) -> p j c", c=6)
                    nc.scalar.activation(lp3[0:pw, :, 0:2], hp3[0:pw, :, 0:2],
                                         AF.Tanh)
                    nc.scalar.activation(lp3[0:pw, :, 2:4], hp3[0:pw, :, 2:4],
                                         AF.Sigmoid)
                    nc.scalar.activation(lp3[0:pw, :, 4:5], hp3[0:pw, :, 4:5],
                                         AF.Copy)
                    xls = xls_pair[half]
                    xb = xl_base + 5 * ((pcol0 + s0) // HD)
                    xv = xls[0:pw, xb:xb + 5 * nj]
                    d = work.tile([HD, 5 * nj], f32, tag="dd", name="dd")
                    d3 = d[:].rearrange("p (j c) -> p j c", c=5)
                    nc.vector.tensor_sub(d3[0:pw, :, :],
                                         xv.rearrange("p (j c) -> p j c", c=5),
                                         lp3[0:pw, :, 0:5])
                    sq = work.tile([HD, 5 * nj], f32, tag="dsq", name="dsq")
                    col = alloc_col(k)
                    nc.scalar.activation(sq[0:pw, :], d[0:pw, :], AF.Square,
                                         accum_out=acc[0:pw, col:col + 1])

        # ================= ENCODER =================
        with tc.tile_pool(name="encarr", bufs=1) as earr, \
             tc.tile_pool(name="encst", bufs=2) as estr:
            xrt = earr.tile([xr_parts, xw], f32, tag="xrt", name="xrt")
            nc.sync.dma_start(xrt[:], d_xraw)

            def xrv(base):
                def xf(s0, sw):
                    g = base + s0
                    s = g // xw
                    assert (g + sw - 1) // xw == s
                    return (xrt[32 * s:32 * s + 5, g % xw:g % xw + sw], s)
                return xf

            eh = {}
            ec = {}
            for j in range(3, loclev + 1):
                n = max(1, lpc >> j)
                eh[j] = earr.tile([HD, n], f32, tag=f"eh{j}", name=f"eh{j}")
                ec[j] = earr.tile([HD, n], f32, tag=f"ec{j}", name=f"ec{j}")

            for c in range(lpc // chw):
                fht = estr.tile([HD, chw], f32, tag="fht", name="fht")
                fct = estr.tile([HD, chw], f32, tag="fct", name="fct")
                nc.sync.dma_start(fht[:], d_fh0[:, c * chw:(c + 1) * chw])
                nc.sync.dma_start(fct[:], d_fc0[:, c * chw:(c + 1) * chw])
                hsrc, csrc = fht[:], fct[:]
                w = chw
                for k in range(3):
                    xf = xrv(xoff[k] + c * (chw >> k))
                    if k < 2:
                        hn = estr.tile([HD, w // 2], f32, tag=f"ehn{k}",
                                       name=f"ehn{k}")[:]
                        cn = estr.tile([HD, w // 2], f32, tag=f"ecn{k}",
                                       name=f"ecn{k}")[:]
                    else:
                        p0 = c * (chw >> 3)
                        pw = chw >> 3
                        hn = eh[3][:, p0:p0 + pw]
                        cn = ec[3][:, p0:p0 + pw]
                    emit_enc_level(hsrc, csrc, xf, w, hn, cn)
                    hsrc, csrc = hn, cn
                    w //= 2

            for k in range(3, loclev):
                n = lpc >> k
                emit_enc_level(eh[k][:], ec[k][:], xrv(xoff[k]), n,
                               eh[k + 1][:], ec[k + 1][:])

            # ship subtree-root feature to the AllGather bounce buffer
            ag_in = dram.tile([2, HD], f32, tag="ag_in", name="ag_in")
            nc.gpsimd.dma_start(ag_in[0:1, :].transpose([1, 0]),
                                eh[loclev][:])
            nc.gpsimd.dma_start(ag_in[1:2, :].transpose([1, 0]),
                                ec[loclev][:])

        # ================= TOP (replicated) =================
        ag_out = dram.tile([2 * NCORES, HD], f32, tag="ag_out", name="ag_out",
                           addr_space="Shared")
        nc.gpsimd.collective_compute(
            "AllGather", ALU.bypass,
            replica_groups=[list(range(NCORES))],
            ins=[ag_in[:].opt()],
            outs=[ag_out[:].opt()],
        )

        with tc.tile_pool(name="toparr", bufs=1) as tarr:
            f14h = tarr.tile([HD, NCORES], f32, tag="f14h", name="f14h")
            f14c = tarr.tile([HD, NCORES], f32, tag="f14c", name="f14c")
            nc.gpsimd.dma_start(f14h[:], ag_out[0::2, :].transpose([1, 0]))
            nc.gpsimd.dma_start(f14c[:], ag_out[1::2, :].transpose([1, 0]))

            th = {loclev: f14h[:]}
            tcv = {loclev: f14c[:]}
            xt_off = {loclev: 0, loclev + 1: 8, loclev + 2: 12, loclev + 3: 14}
            for j in range(loclev, loclev + 3):
                n = NCORES >> (j - loclev)
                hn = tarr.tile([HD, n // 2], f32, tag=f"th{j}", name=f"th{j}")
                cn = tarr.tile([HD, n // 2], f32, tag=f"tc{j}", name=f"tc{j}")
                emit_enc_level(
                    th[j], tcv[j],
                    lambda s0, sw, b=xt_off[j]: (xtraw[0:5, b + s0:b + s0 + sw], 0),
                    n, hn[:], cn[:])
                th[j + 1] = hn[:]
                tcv[j + 1] = cn[:]

            # top decoder: steps loclev+2, loclev+1, loclev
            sfh, sfc = th[loclev + 3], tcv[loclev + 3]
            for k in range(loclev + 2, loclev - 1, -1):
                P = 1 << (loclev + 2 - k)
                n = 2 * P
                ofh = tarr.tile([HD, n], f32, tag=f"dth{k}", name=f"dth{k}")[:]
                ofc = tarr.tile([HD, n], f32, tag=f"dtc{k}", name=f"dtc{k}")[:]
                emit_dec_step(
                    sfh, sfc,
                    lambda s0, sw, b=xt_off[k + 1]: (xtab[0:5, b + s0:b + s0 + sw], 0),
                    k, 0, P, ofh, ofc,
                    (xtlsl, xtlsr), 5 * (k - loclev), last=False)
                sfh, sfc = ofh, ofc

            # select this core's subtree column with the one-hot mask
            selh = tarr.tile([HD, NCORES], f32, tag="selh", name="selh")
            nc.vector.tensor_mul(selh[:], sfh, mask[:])
            rfh = tarr.tile([HD, 1], f32, tag="rfh", name="rfh")
            nc.vector.tensor_reduce(rfh[:], selh[:], axis=mybir.AxisListType.X,
                                    op=ALU.add)
            selc = tarr.tile([HD, NCORES], f32, tag="selc", name="selc")
            nc.vector.tensor_mul(selc[:], sfc, mask[:])
            rfc = tarr.tile([HD, 1], f32, tag="rfc", name="rfc")
            nc.vector.tensor_reduce(rfc[:], selc[:], axis=mybir.AxisListType.X,
                                    op=ALU.add)

            # ================= LOCAL DECODER =================
            with tc.tile_pool(name="decarr", bufs=1) as darr:
                xat = darr.tile([xa_parts, xaw], f32, tag="xat", name="xat")
                nc.sync.dma_start(xat[:], d_xab)

                def xav(base):
                    def xf(s0, sw):
                        g = base + s0
                        s = g // xaw
                        assert (g + sw - 1) // xaw == s
                        return (xat[32 * s:32 * s + 5,
                                    g % xaw:g % xaw + sw], s)
                    return xf

                lo_arr = min(3, loclev)
                dh = {}
                dc = {}
                for j in range(lo_arr, loclev):
                    n = lpc >> j
                    dh[j] = darr.tile([HD, n], f32, tag=f"dh{j}",
                                      name=f"dh{j}")[:]
                    dc[j] = darr.tile([HD, n], f32, tag=f"dc{j}",
                                      name=f"dc{j}")[:]

                sfh, sfc = rfh[:], rfc[:]
                for k in range(loclev - 1, lo_arr - 1, -1):
                    P = lpc >> (k + 1)
                    lastk = (k == 0)
                    ofh = dh.get(k)
                    ofc = dc.get(k)
                    emit_dec_step(sfh, sfc, xav(xpoff[k + 1]),
                                  k, 0, P, ofh, ofc,
                                  (xlsl, xlsr), xloff[k], last=lastk)
                    sfh, sfc = ofh, ofc

                if lo_arr > 0:
                    Ptop = lpc >> lo_arr
                    cpw = min(256, Ptop)
                    with tc.tile_pool(name="decf", bufs=2) as dfp:
                        for c in range(Ptop // cpw):
                            cfh = sfh[:, c * cpw:(c + 1) * cpw]
                            cfc = sfc[:, c * cpw:(c + 1) * cpw]
                            for k in range(lo_arr - 1, -1, -1):
                                P = cpw << (lo_arr - 1 - k)
                                pc0 = c * cpw << (lo_arr - 1 - k)
                                lastk = (k == 0)
                                if not lastk:
                                    ofh = dfp.tile([HD, 2 * P], f32,
                                                   tag=f"fh{k}",
                                                   name=f"fh{k}")[:]
                                    ofc = dfp.tile([HD, 2 * P], f32,
                                                   tag=f"fc{k}",
                                                   name=f"fc{k}")[:]
                                else:
                                    ofh = ofc = None
                                emit_dec_step(
                                    cfh, cfc, xav(xpoff[k + 1] + pc0),
                                    k, pc0, P, ofh, ofc,
                                    (xlsl, xlsr), xloff[k], last=lastk)
                                cfh, cfc = ofh, ofc

        nc.sync.dma_start(d_out, acc[:])

    nc.compile()
    return nc, col_map


# ---------------------------------------------------------------------------
# host-side input preparation
# ---------------------------------------------------------------------------

def _xab_host(X, TL, nlev, offs):
    """Absolute-frame positions for every node (pure function of X)."""
    X_ab = X.astype(F32).copy()
    for k in range(nlev - 1, -1, -1):
        sz = TL >> k
        ch = np.arange(offs[k], offs[k] + sz)
        par = np.arange(offs[k + 1], offs[k + 1] + sz // 2)
        pp = X_ab[np.repeat(par, 2)]  # parent per child
        C = X_ab[ch]
        X_ab[ch, 0:2] = C[:, 0:2] * pp[:, 2:4] + pp[:, 0:2]
        X_ab[ch, 2:4] = C[:, 2:4] * pp[:, 2:4]
        X_ab[ch, 4] = C[:, 4] + pp[:, 4]
    return X_ab


def _xloss_host(Xside, m):
    """(m, 5) child X -> (128, 5*ceil(m/128)) block-major loss layout."""
    blocks = (m + 127) // 128
    out = np.zeros((blocks * 128, 5), F32)
    out[:m] = Xside
    return out.reshape(blocks, 128, 5).transpose(1, 0, 2).reshape(128, -1)


def prepare_inputs(inputs, lpc):
    X = np.ascontiguousarray(np.asarray(inputs["X"], F32))
    Feat = np.asarray(inputs["Feature"], F32)

    (loclev, xoff, xin_pad, xpoff, xab_pad, xloff, xloss_cols,
     xw, xaw) = _tree_meta(lpc)
    nlev = loclev + 3
    TL = lpc * NCORES
    sizes = [TL >> j for j in range(nlev + 1)]
    offs = np.cumsum([0] + sizes)

    X_ab = _xab_host(X, TL, nlev, offs)

    w = {}
    enc_Wih = np.asarray(inputs["enc_Wih"], F32)
    enc_Whh = np.asarray(inputs["enc_Whh"], F32)
    ewih_t = np.ascontiguousarray(enc_Wih.T)
    ewih4 = np.zeros((101, 4 * HD), F32)
    for s in range(4):
        ewih4[32 * s:32 * s + 5] = ewih_t
    w["ewih"] = ewih4
    w["ewhh"] = np.ascontiguousarray(enc_Whh.T)
    ebv = (np.asarray(inputs["enc_bih"], F32)
           + np.asarray(inputs["enc_bhh"], F32))
    w["eb"] = np.ascontiguousarray(ebv.reshape(4, HD).T)
    fc_h_W = np.asarray(inputs["fc_h_W"], F32)
    w["fha"] = np.ascontiguousarray(fc_h_W[:, 0:HD].T)
    w["fhb"] = np.ascontiguousarray(fc_h_W[:, HD:NF].T)
    w["fhbias"] = np.ascontiguousarray(
        np.asarray(inputs["fc_h_b"], F32).reshape(4, HD).T)
    dec_Wih = np.asarray(inputs["dec_Wih"], F32)
    dwx_t = np.ascontiguousarray(dec_Wih[:, 0:5].T)
    dwx4 = np.zeros((101, 4 * NF), F32)
    for s in range(4):
        dwx4[32 * s:32 * s + 5] = dwx_t
    w["dwx"] = dwx4
    w["dwfh"] = np.ascontiguousarray(dec_Wih[:, 5:5 + HD].T)
    w["dwfl"] = np.ascontiguousarray(dec_Wih[:, 5 + HD:5 + NF].T)
    dec_Whh = np.asarray(inputs["dec_Whh"], F32)
    w["dha"] = np.ascontiguousarray(dec_Whh[:, 0:HD].T)
    w["dhb"] = np.ascontiguousarray(dec_Whh[:, HD:NF].T)
    dbv = (np.asarray(inputs["dec_bih"], F32)
           + np.asarray(inputs["dec_bhh"], F32))
    w["db"] = np.ascontiguousarray(dbv.reshape(8, HD).T)
    w["flw"] = np.ascontiguousarray(np.asarray(inputs["fc_l_W"], F32).T)
    w["frw"] = np.ascontiguousarray(np.asarray(inputs["fc_r_W"], F32).T)
    w["hbias"] = np.concatenate(
        [np.asarray(inputs["fc_l_b"], F32),
         np.asarray(inputs["fc_r_b"], F32)]).reshape(1, 12)

    # top-node arrays (same for all cores): the last 15 nodes
    topx = np.zeros((16, 5), F32)
    topx[:15] = X[offs[loclev]:offs[loclev] + 15]
    w["xtraw"] = np.ascontiguousarray(topx.T)
    topab = np.zeros((16, 5), F32)
    topab[:15] = X_ab[offs[loclev]:offs[loclev] + 15]
    w["xtab"] = np.ascontiguousarray(topab.T)
    # top loss slabs: levels loclev..loclev+2, merges 4, 2, 1 (global)
    xtl = np.zeros((HD, 16), F32)
    xtr = np.zeros((HD, 16), F32)
    for t in range(3):
        k = loclev + t
        m = sizes[k + 1]  # merges at level k (4, 2, 1)
        ch = offs[k]
        xtl[:, 5 * t:5 * t + 5] = _xloss_host(X[ch:ch + 2 * m:2], m)
        xtr[:, 5 * t:5 * t + 5] = _xloss_host(X[ch + 1:ch + 2 * m:2], m)
    w["xtlsl"] = xtl
    w["xtlsr"] = xtr

    in_maps = []
    for core in range(NCORES):
        m = dict(w)
        lsl = slice(core * lpc, (core + 1) * lpc)
        m["fh0"] = np.ascontiguousarray(Feat[lsl, 0:HD].T)
        m["fc0"] = np.ascontiguousarray(Feat[lsl, HD:NF].T)
        xs = []
        for k in range(loclev):
            nk = lpc >> k
            base = offs[k] + core * nk
            xs.append(X[base:base + nk])
        xcat = np.concatenate(xs, axis=0)
        if xin_pad != xcat.shape[0]:
            xcat = np.concatenate(
                [xcat, np.zeros((xin_pad - xcat.shape[0], 5), F32)], axis=0)
        m["xraw"] = _pack_x(xcat.T, xw)
        xs = []
        for k in range(1, loclev + 1):
            nk = lpc >> k
            base = offs[k] + core * nk
            xs.append(X_ab[base:base + nk])
        xpc = np.concatenate(xs, axis=0)
        if xab_pad != xpc.shape[0]:
            xpc = np.concatenate(
                [xpc, np.zeros((xab_pad - xpc.shape[0], 5), F32)], axis=0)
        m["xab"] = _pack_x(xpc.T, xaw)
        ll = np.zeros((HD, xloss_cols), F32)
        rr = np.zeros((HD, xloss_cols), F32)
        for k in range(loclev):
            mk = lpc >> (k + 1)
            base = offs[k] + core * (lpc >> k)
            wk = 5 * ((mk + 127) // 128)
            ll[:, xloff[k]:xloff[k] + wk] = _xloss_host(
                X[base:base + 2 * mk:2], mk)
            rr[:, xloff[k]:xloff[k] + wk] = _xloss_host(
                X[base + 1:base + 2 * mk:2], mk)
        m["xlsl"] = ll
        m["xlsr"] = rr
        mk_ = np.zeros((HD, NCORES), F32)
        mk_[:, core] = 1.0
        m["mask"] = mk_
        in_maps.append(m)
    return in_maps


def combine_outputs(results, col_map, lpc):
    loclev = int(math.log2(lpc))
    nlev = loclev + 3
    TL = lpc * NCORES
    sse = np.zeros(nlev, np.float64)
    for core in range(NCORES):
        a = np.asarray(results[core]["acc_out"], np.float64)
        for col, k in enumerate(col_map):
            if k < loclev or core == 0:
                sse[k] += a[:, col].sum()
    loss = 0.0
    for k in range(nlev):
        mk = TL >> (k + 1)
        loss += sse[k] / (5.0 * mk)
    return loss / nlev


# ---------------------------------------------------------------------------
# top-level kernel
# ---------------------------------------------------------------------------

_PROG_CACHE = {}
last_exec_time_ns = None


def _get_program(lpc, **kw):
    key = (lpc, tuple(sorted(kw.items())))
    if key not in _PROG_CACHE:
        _PROG_CACHE[key] = build_program(lpc, **kw)
    return _PROG_CACHE[key]


def kernel(**inputs):
    global last_exec_time_ns
    import os

    from concourse.bass_utils import run_bass_kernel_spmd

    kw = {}
    mm_dt = os.environ.get("AE_MM_DT") or None
    if mm_dt:
        kw["mm_dt"] = mm_dt
    nc, col_map = _get_program(FULL_LPC, **kw)
    in_maps = prepare_inputs(inputs, FULL_LPC)
    trace = bool(os.environ.get("AE_TRACE"))
    res = run_bass_kernel_spmd(nc, in_maps, core_ids=list(range(NCORES)),
                               trace=trace)
    last_exec_time_ns = getattr(res, "exec_time_ns", None)
    loss = combine_outputs(res.results, col_map, FULL_LPC)
    return np.float32(loss)
